# revision 14
# baseline (speedup 1.0000x reference)
"""CenterLoss on 8 TRN2 NeuronCores (Bass kernel, data-parallel over batch).

Problem (fixed shapes, fp32):
    x       [4096, 2048]   features
    labels  [4096]         int    (class ids in [0, 6625))
    centers [6625, 2048]   class centers

    loss = mean_i( clip( ||x_i - centers[labels_i]||^2, 1e-12, 1e12 ) )

Sharding: batch split 512 rows/core across 8 cores; centers replicated
(stay in DRAM - only the 512 labeled rows are gathered per core via
indirect DMA). Each core returns its partial sum of clamped squared
distances; the host sums the 8 partials and divides by 4096.

Per-core pipeline (raw Bass, manual semaphores):
    sync    : labels DMA, then 4x 1MiB x-tile DMAs (HWDGE)
    gpsimd  : 4x indirect gather centers[labels] -> SBUF (SWDGE), 1MiB each
    vector  : diff = x - c per tile (DVE)
    scalar  : Square activation with accum_out => per-row sum((x-c)^2) (ACT)
    vector  : clamp [1e-12, 1e12] + reduce over tiles -> [128,1]
    tensor  : ones[128,1].T @ red[128,1] -> partition-sum in PSUM
    scalar  : PSUM -> SBUF copy
    sync    : result DMA out
"""

from contextlib import ExitStack

import numpy as np

import concourse.bass as bass
import concourse.mybir as mybir
from concourse.bass_utils import run_bass_kernel_spmd

BATCH = 4096
FEAT = 2048
NCLASSES = 6625
NCORES = 8
SHARD = BATCH // NCORES  # 512 rows per core
P = 128                  # partitions
NT = SHARD // P          # 4 tiles of [128, FEAT] per core
F32 = mybir.dt.float32


def build_bass():
    nc = bass.Bass("TRN2", target_bir_lowering=False, debug=False)

    x = nc.dram_tensor("x", [SHARD, FEAT], F32, kind="ExternalInput")
    # labels pre-arranged host-side to [128, NT]: labels_pn[p, n] = labels[n*128+p]
    labels = nc.dram_tensor("labels", [P, NT], mybir.dt.int32, kind="ExternalInput")
    centers = nc.dram_tensor("centers", [NCLASSES, FEAT], F32, kind="ExternalInput")
    out = nc.dram_tensor("out", [1, 1], F32, kind="ExternalOutput")

    with ExitStack() as stack:
        sb = lambda *a: stack.enter_context(nc.sbuf_tensor(*a))
        sem = lambda name: stack.enter_context(nc.semaphore(name))

        xt = sb("xt", [P, NT * FEAT], F32)
        ct = sb("ct", [P, NT * FEAT], F32)
        # write-only dumps for the fused ops' elementwise outputs
        scr1 = sb("scr1", [P, NT * FEAT], F32)
        scr2 = sb("scr2", [P, NT * FEAT], F32)
        scrD = sb("scrD", [P, NT * FEAT], F32)
        lab = sb("lab", [P, NT], mybir.dt.int32)
        xsq = sb("xsq", [P, NT], F32)      # per-row sum x^2
        csq = sb("csq", [P, NT], F32)      # per-row sum c^2
        cr2 = sb("cr2", [P, NT], F32)      # per-row -2*sum(x*c)
        t1 = sb("t1", [P, NT], F32)
        dist = sb("dist", [P, NT], F32)
        red = sb("red", [P, 1], F32)
        ones = sb("ones", [P, 1], F32)
        out_sb = sb("out_sb", [1, 1], F32)
        acc = stack.enter_context(nc.psum_tensor("acc", [1, 1], F32))

        labsem = sem("labsem")   # labels DMA
        outsem = sem("outsem")   # result DMA
        vsem = sem("vsem")       # DVE cross ttr per tile
        asem = sem("asem")       # ACT square+accum (8 total)
        vsem2 = sem("vsem2")     # DVE final chain done
        s1 = sem("s1")           # DVE self-sync
        s2 = sem("s2")           # DVE self-sync
        s3 = sem("s3")           # DVE self-sync
        msem = sem("msem")       # PE matmul done
        osem = sem("osem")       # result in out_sb
        # one sem per DMA: concurrent DMAs on one sem can't be gated by
        # cumulative thresholds (per-engine completions interleave)
        xsem = [stack.enter_context(nc.semaphore(f"xsem{n}")) for n in range(NT)]
        csem = [stack.enter_context(nc.semaphore(f"csem{n}")) for n in range(NT)]
        block = stack.enter_context(nc.Block())

        @block.sync
        def _(sync):
            sync.dma_start(out=lab[:, :], in_=labels[:, :]).then_inc(labsem, 16)
            for n in range(NT):
                sync.dma_start(
                    out=xt[:, n * FEAT:(n + 1) * FEAT],
                    in_=x[n * P:(n + 1) * P, :],
                ).then_inc(xsem[n], 16)
            sync.wait_ge(osem, 1)
            sync.dma_start(out=out[:, :], in_=out_sb[:, :]).then_inc(outsem, 16)
            sync.wait_ge(outsem, 16)

        @block.gpsimd
        def _(gpsimd):
            gpsimd.wait_ge(labsem, 16)  # labels landed
            for n in range(NT):
                gpsimd.indirect_dma_start(
                    out=ct[:, n * FEAT:(n + 1) * FEAT],
                    out_offset=None,
                    in_=centers[:, :],
                    in_offset=bass.IndirectOffsetOnAxis(ap=lab[:, n:n + 1], axis=0),
                ).then_inc(csem[n], 16)

        @block.scalar
        def _(scalar):
            # x^2 row-sums as soon as each x tile lands (before gathers finish)
            for n in range(NT):
                fsl = slice(n * FEAT, (n + 1) * FEAT)
                scalar.wait_ge(xsem[n], 16)
                scalar.activation(
                    out=scr1[:, fsl], in_=xt[:, fsl],
                    func=mybir.ActivationFunctionType.Square,
                    accum_out=xsq[:, n:n + 1],
                ).then_inc(asem, 1)
            # c^2 row-sums as each gather lands
            for n in range(NT):
                fsl = slice(n * FEAT, (n + 1) * FEAT)
                scalar.wait_ge(csem[n], 16)
                scalar.activation(
                    out=scr2[:, fsl], in_=ct[:, fsl],
                    func=mybir.ActivationFunctionType.Square,
                    accum_out=csq[:, n:n + 1],
                ).then_inc(asem, 1)
            scalar.wait_ge(msem, 1)
            scalar.copy(out=out_sb[:, :], in_=acc[:, :]).then_inc(osem, 1)

        @block.vector
        def _(vector):
            vector.memset(ones[:, :], 1.0)
            # -2 * <x_i, c_i> per row, fused multiply+reduce (runs || to ACT)
            for n in range(NT):
                fsl = slice(n * FEAT, (n + 1) * FEAT)
                vector.wait_ge(xsem[n], 16)
                vector.wait_ge(csem[n], 16)
                vector.scalar_tensor_tensor(
                    out=scrD[:, fsl], in0=xt[:, fsl], scalar=-2.0, in1=ct[:, fsl],
                    op0=mybir.AluOpType.mult, op1=mybir.AluOpType.mult,
                    accum_out=cr2[:, n:n + 1],
                ).then_inc(vsem, 1)
            # dist = clip(xsq + csq + cr2); red = per-partition sum
            vector.wait_ge(asem, 2 * NT)
            vector.tensor_add(out=t1[:, :], in0=xsq[:, :], in1=csq[:, :]).then_inc(s1, 1)
            vector.wait_ge(s1, 1)
            vector.wait_ge(vsem, NT)
            vector.tensor_add(out=dist[:, :], in0=t1[:, :], in1=cr2[:, :]).then_inc(s2, 1)
            vector.wait_ge(s2, 1)
            vector.tensor_scalar(
                out=dist[:, :], in0=dist[:, :],
                scalar1=1e-12, scalar2=1e12,
                op0=mybir.AluOpType.max, op1=mybir.AluOpType.min,
            ).then_inc(s3, 1)
            vector.wait_ge(s3, 1)
            vector.reduce_sum(
                out=red[:, :], in_=dist[:, :], axis=mybir.AxisListType.X
            ).then_inc(vsem2, 1)

        @block.tensor
        def _(tensor):
            tensor.wait_ge(vsem2, 1)
            tensor.matmul(
                out=acc[:, :], lhsT=ones[:, :], rhs=red[:, :],
                start=True, stop=True,
            ).then_inc(msem, 1)

    return nc


def make_in_maps(x, labels, centers):
    """Shard full inputs into per-core input maps (data-parallel over batch)."""
    x = np.ascontiguousarray(np.asarray(x, dtype=np.float32))
    labels_i32 = np.ascontiguousarray(np.asarray(labels).astype(np.int32))
    centers = np.ascontiguousarray(np.asarray(centers, dtype=np.float32))
    assert x.shape == (BATCH, FEAT) and centers.shape == (NCLASSES, FEAT)
    assert labels_i32.shape == (BATCH,)
    return [
        {
            "x": x[c * SHARD:(c + 1) * SHARD],
            # [SHARD] -> [128, NT] with lab[p, n] = labels[n*128 + p]
            "labels": np.ascontiguousarray(
                labels_i32[c * SHARD:(c + 1) * SHARD].reshape(NT, P).T
            ),
            "centers": centers,
        }
        for c in range(NCORES)
    ]


def kernel(x, labels, centers):
    nc = build_bass()
    in_maps = make_in_maps(x, labels, centers)
    res = run_bass_kernel_spmd(nc, in_maps, core_ids=list(range(NCORES)))
    total = float(sum(float(r["out"][0, 0]) for r in res.results))
    return np.float32(total / BATCH)


if __name__ == "__main__":
    rng = np.random.default_rng(0)
    x = rng.standard_normal((BATCH, FEAT), dtype=np.float32)
    labels = rng.integers(0, NCLASSES, size=(BATCH,)).astype(np.int32)
    centers = rng.standard_normal((NCLASSES, FEAT), dtype=np.float32)
    got = kernel(x=x, labels=labels, centers=centers)
    c = centers[labels]
    d = ((x - c) ** 2).sum(axis=1)
    want = np.clip(d, 1e-12, 1e12).mean()
    print("kernel:", got, "numpy:", want, "rel:", abs(got - want) / abs(want))
